# revision 22
# baseline (speedup 1.0000x reference)
"""CrissCrossAttention (multi-scale dilated conv + criss-cross axial attention)
Trainium2 Bass/Tile kernel, 8 NeuronCores.

Sharding: 8 cores = 4 batch samples x 2 H-halves.

v3 design:
 - conv as 25 folded taps in fp8e4 with DoubleRow (K=256 per matmul), flat
   416-col rhs runs spanning 4 padded rows (halo cols never read back).
 - all projections (q/k/vT) in fp8 DoubleRow off an fp8 ms tensor.
 - energies computed TRANSPOSED (source index on partitions) so no per-line
   transposes are needed; exp batched 5-10 lines per scalar-engine call.
 - softmax denominators ride along as a ones-column appended to the vT
   operands of the apply matmuls; joint normalization deferred to the tail.
 - pair exchange: vT (fp8) first then k (bf16) via AllGather; a tiny warmup
   collective during conv absorbs the CC-engine startup latency.
 - 16B-aligned strides (272/264) for all hot attention tiles.
 - col->row layout merge via chunked DRAM bounce overlapped with col apply.
 - output kept in [w, h, c] layout on chip; host transposes for free.
"""

import numpy as np
import ml_dtypes

BF16 = ml_dtypes.bfloat16
F8 = ml_dtypes.float8_e4m3

B, C, H, W = 4, 256, 96, 96
CQ = 32
HC = 48              # rows per core
NPOS = HC * W        # 4608 positions per core
HP, WP = 58, 104     # padded slab: 3+48+3 halo rows +4 slack, 3+96+5 cols
NCORES = 8
SW = 64.0            # weight scale for fp8
SV = 32.0            # v scale for fp8
VS = 272             # padded stride of vT tiles (16B aligned, >=257)
AS = 264             # padded stride of acc tiles (bf16 -> 528B, 16B aligned)

NROW = 4             # image rows per conv N-tile
CONV_N = NROW * WP   # 416 flat cols per conv matmul (incl junk)
NT_PROJ = 9
PROJ_N = 512
RG = [[0, 1], [2, 3], [4, 5], [6, 7]]


def _fold_taps(w_ms):
    taps = {}
    for i, d in enumerate((1, 2, 3)):
        for iy in range(3):
            for ix in range(3):
                off = ((iy - 1) * d, (ix - 1) * d)
                if off in taps:
                    taps[off] = taps[off] + w_ms[i][:, :, iy, ix]
                else:
                    taps[off] = w_ms[i][:, :, iy, ix].copy()
    offs = sorted(taps)
    assert len(offs) == 25
    return offs, taps


def _build_program(gamma_f, offs):
    import concourse.mybir as mybir
    import concourse.tile as tile
    from concourse import bacc
    from concourse.masks import make_identity

    dt = mybir.dt
    DR = mybir.MatmulPerfMode.DoubleRow
    nc = bacc.Bacc("TRN2", target_bir_lowering=False, debug=False,
                   num_devices=NCORES)

    xpad_d = nc.dram_tensor("xpad", [128, 2, HP, WP], dt.float8e4, kind="ExternalInput")
    w25_d = nc.dram_tensor("w25", [128, 25, 2, 2, 128], dt.float8e4, kind="ExternalInput")
    wq_d = nc.dram_tensor("wq8", [128, 2, CQ], dt.float8e4, kind="ExternalInput")
    wk_d = nc.dram_tensor("wk8", [128, 2, CQ], dt.float8e4, kind="ExternalInput")
    wv_d = nc.dram_tensor("wv8", [128, 2, 256], dt.float8e4, kind="ExternalInput")
    bq_d = nc.dram_tensor("bq", [CQ, 1], dt.float32, kind="ExternalInput")
    bk_d = nc.dram_tensor("bk", [CQ, 1], dt.float32, kind="ExternalInput")
    bsum_d = nc.dram_tensor("bsum", [2, 128, 1], dt.float32, kind="ExternalInput")
    mask_d = nc.dram_tensor("mask01", [96, NPOS], dt.bfloat16, kind="ExternalInput")
    xres_d = nc.dram_tensor("xres", [2, 128, NPOS], dt.float32, kind="ExternalInput")
    out_d = nc.dram_tensor("out", [2, 128, NPOS], dt.float32, kind="ExternalOutput")

    with tile.TileContext(nc) as tc:
        with (
            tc.tile_pool(name="const", bufs=1) as constp,
            tc.tile_pool(name="dram", bufs=1, space="DRAM") as dramp,
            tc.tile_pool(name="persist", bufs=1) as pp,
        ):
            # ---- constants ----
            id48 = constp.tile([HC, HC], dt.bfloat16, tag="id48", name="id48")
            make_identity(nc, id48)
            bq_sb = constp.tile([CQ, 1], dt.float32, tag="bq", name="bq_sb")
            nc.scalar.dma_start(out=bq_sb, in_=bq_d[:])
            bk_sb = constp.tile([CQ, 1], dt.float32, tag="bk", name="bk_sb")
            nc.scalar.dma_start(out=bk_sb, in_=bk_d[:])
            bsum_sb = [constp.tile([128, 1], dt.float32, tag=f"bs{m}", name=f"bsum{m}")
                       for m in range(2)]
            for m in range(2):
                nc.scalar.dma_start(out=bsum_sb[m], in_=bsum_d[m])

            # ---- persistent tensors ----
            kf = pp.tile([CQ, 2, NPOS], dt.bfloat16, tag="kf", name="kf")
            k_own = pp.tile([CQ, NPOS], dt.bfloat16, tag="ko", name="k_own")
            q_sb = pp.tile([CQ, NPOS], dt.bfloat16, tag="q", name="q_sb")
            vTa = pp.tile([96, HC, 256], dt.float8e4, tag="vTa", name="vTa")
            vTb = pp.tile([96, 96, 256], dt.float8e4, tag="vTb", name="vTb")
            # attention accumulators, c on partitions; accR is (h,w)-major,
            # accC is (w,h)-major (each written contiguously by its apply)
            accR = [pp.tile([128, NPOS], dt.bfloat16, tag=f"accR{m}",
                            name=f"accR{m}") for m in range(2)]
            accC = [pp.tile([128, NPOS], dt.bfloat16, tag=f"accC{m}",
                            name=f"accC{m}") for m in range(2)]
            recipDd = pp.tile([128, NPOS], dt.bfloat16, tag="rDd",
                              name="recipDd")
            onesD = pp.tile([96, 128], dt.bfloat16, tag="o1", name="onesD")
            attWT = pp.tile([96, NPOS], dt.bfloat16, tag="awt", name="attWT")
            nc.vector.memset(onesD[:], SV / gamma_f)

            # ---- dram bounce buffers ----
            pack_k = dramp.tile([CQ, NPOS], dt.bfloat16, tag="pk", name="pack_k")
            pack_v = dramp.tile([HC, 96, 256], dt.float8e4, tag="pv", name="pack_v")
            gath_k = dramp.tile([2, CQ, NPOS], dt.bfloat16, tag="gk", name="gath_k")
            gath_v = dramp.tile([2, HC, 96, 256], dt.float8e4, tag="gv", name="gath_v")
            warm_i = dramp.tile([1, 48], dt.bfloat16, tag="wi", name="warm_i")
            warm_o = dramp.tile([2, 1, 48], dt.bfloat16, tag="wo", name="warm_o")

            # warmup collective: absorbs the ~11us CC startup latency while
            # the conv runs.
            nc.gpsimd.dma_start(out=warm_i[:], in_=id48[0:1, 0:48])
            nc.gpsimd.collective_compute(
                "AllGather", mybir.AluOpType.bypass, replica_groups=RG,
                ins=[warm_i[:]], outs=[warm_o[:]])

            # ================= Phase 1: conv (25 taps, fp8 DoubleRow) ========
            msp_ctx = tc.tile_pool(name="msp", bufs=1)
            msp = msp_ctx.__enter__()
            ms8 = msp.tile([128, 2, NPOS], dt.float8e4, tag="ms8", name="ms8")
            with (
                tc.tile_pool(name="xw", bufs=1) as xwp,
                tc.tile_pool(name="cvps", bufs=1, space="PSUM") as cvps,
            ):
                w25_sb = xwp.tile([128, 25, 2, 2, 128], dt.float8e4, tag="wt",
                                  name="w25_sb")
                nc.gpsimd.dma_start(out=w25_sb[:, 0:13], in_=w25_d[:, 0:13])
                nc.gpsimd.dma_start(out=w25_sb[:, 13:25], in_=w25_d[:, 13:25])
                xpad_sb = xwp.tile([128, 2, HP, WP], dt.float8e4, tag="xp",
                                   name="xpad_sb")
                nc.sync.dma_start(out=xpad_sb[:, :, 0:26], in_=xpad_d[:, :, 0:26])
                nc.sync.dma_start(out=xpad_sb[:, :, 26:HP], in_=xpad_d[:, :, 26:HP])
                xflat = xpad_sb.rearrange("p k h w -> p k (h w)")

                for g in range(3):      # 3 groups of (2m x 4j) psum tiles
                    P = [[cvps.tile([128, CONV_N], dt.float32, tag=f"cv{m}{j}",
                                    name=f"P{g}{m}{j}", bufs=1)
                          for j in range(4)] for m in range(2)]
                    for t in range(25):
                        dy, dx = offs[t]
                        for m in range(2):
                            lhsT = w25_sb[:, t, :, m, :]
                            for j in range(4):
                                nj = g * 4 + j
                                base = (nj * NROW + 3 + dy) * WP + 3 + dx
                                rhs = xflat[:, :, base:base + CONV_N]
                                nc.tensor.matmul(P[m][j], lhsT, rhs,
                                                 start=(t == 0), stop=(t == 24),
                                                 perf_mode=DR)
                    for m in range(2):
                        for j in range(4):
                            nj = g * 4 + j
                            dst = ms8[:, m, nj * NROW * W:(nj + 1) * NROW * W]
                            nc.vector.tensor_scalar(
                                out=dst.rearrange("p (r c) -> p r c", c=W),
                                in0=P[m][j].rearrange("p (r c) -> p r c", c=WP)[:, :, 0:W],
                                scalar1=1.0 / SW, scalar2=bsum_sb[m],
                                op0=mybir.AluOpType.mult,
                                op1=mybir.AluOpType.add)

            # ================= Phase 2: projections (fp8 DoubleRow) =========
            with (
                tc.tile_pool(name="wproj", bufs=1) as wpp,
                tc.tile_pool(name="pjps", bufs=1, space="PSUM") as pjps,
            ):
                wv_sb = wpp.tile([128, 2, 256], dt.float8e4, tag="wv", name="wv_sb")
                nc.sync.dma_start(out=wv_sb, in_=wv_d[:])
                wq_sb = wpp.tile([128, 2, CQ], dt.float8e4, tag="wq", name="wq_sb")
                nc.sync.dma_start(out=wq_sb, in_=wq_d[:])
                wk_sb = wpp.tile([128, 2, CQ], dt.float8e4, tag="wk", name="wk_sb")
                nc.sync.dma_start(out=wk_sb, in_=wk_d[:])

                # k projection first: its gather is latency-bound and the
                # column energies need it earliest.
                for n in range(NT_PROJ):
                    sl = slice(n * PROJ_N, (n + 1) * PROJ_N)
                    pk = pjps.tile([CQ, PROJ_N], dt.float32, tag="pjk",
                                   name=f"pk{n}", bufs=2)
                    nc.tensor.matmul(pk, wk_sb, ms8[:, :, sl], start=True,
                                     stop=True, perf_mode=DR)
                    nc.vector.tensor_scalar(out=k_own[:, sl], in0=pk,
                                            scalar1=1.0 / SW, scalar2=bk_sb,
                                            op0=mybir.AluOpType.mult,
                                            op1=mybir.AluOpType.add)
                nc.gpsimd.dma_start(out=pack_k[:], in_=k_own[:])
                nc.gpsimd.collective_compute(
                    "AllGather", mybir.AluOpType.bypass, replica_groups=RG,
                    ins=[pack_k[:]], outs=[gath_k[:]])
                for gi in range(2):
                    nc.sync.dma_start(out=kf[:, gi, :], in_=gath_k[gi])

                # vT projection + the big (bandwidth-bound) exchange.
                for h in range(HC):
                    pv = pjps.tile([96, 256], dt.float32, tag="pjv",
                                   name=f"pv{h}", bufs=4)
                    nc.tensor.matmul(pv, ms8[:, :, h * W:(h + 1) * W], wv_sb,
                                     start=True, stop=True, perf_mode=DR)
                    nc.vector.tensor_scalar_mul(out=vTa[:, h, :], in0=pv,
                                                scalar1=SV / SW)
                nc.gpsimd.dma_start(
                    out=pack_v[:].rearrange("h w c -> w h c"),
                    in_=vTa[:, :, 0:256])
                nc.gpsimd.collective_compute(
                    "AllGather", mybir.AluOpType.bypass, replica_groups=RG,
                    ins=[pack_v[:]], outs=[gath_v[:]])
                for gi in range(2):
                    eng = nc.sync if gi == 0 else nc.gpsimd
                    for wh in range(2):
                        eng.dma_start(
                            out=vTb[gi * HC:(gi + 1) * HC,
                                    wh * HC:(wh + 1) * HC, :],
                            in_=gath_v[gi, :, wh * HC:(wh + 1) * HC, :])

                # q projection
                for n in range(NT_PROJ):
                    sl = slice(n * PROJ_N, (n + 1) * PROJ_N)
                    pq = pjps.tile([CQ, PROJ_N], dt.float32, tag="pjq",
                                   name=f"pq{n}", bufs=2)
                    nc.tensor.matmul(pq, wq_sb, ms8[:, :, sl], start=True,
                                     stop=True, perf_mode=DR)
                    nc.vector.tensor_scalar(out=q_sb[:, sl], in0=pq,
                                            scalar1=1.0 / SW, scalar2=bq_sb,
                                            op0=mybir.AluOpType.mult,
                                            op1=mybir.AluOpType.add)

            msp_ctx.__exit__(None, None, None)

            # ============ Phase 3: row attention (fully local) ==============
            HB = [5] * 9 + [3]          # 48 h in blocks of 5 (plus tail 3)
            with (
                tc.tile_pool(name="rps", bufs=1, space="PSUM") as rps,
            ):
                h = 0
                for nb in HB:
                    pew = rps.tile([96, nb * 96], dt.float32, tag="pew",
                                   name=f"pew{h}", bufs=2)
                    for i in range(nb):
                        sl = slice((h + i) * 96, (h + i + 1) * 96)
                        nc.tensor.matmul(pew[:, i * 96:(i + 1) * 96],
                                         k_own[:, sl], q_sb[:, sl],
                                         start=True, stop=True)
                    nc.scalar.activation(
                        out=attWT[:, h * 96:(h + nb) * 96], in_=pew,
                        func=mybir.ActivationFunctionType.Exp)
                    h += nb
                for h in range(HC):
                    for m in range(2):
                        po = rps.tile([128, 96], dt.float32, tag="po",
                                      name=f"po{h}{m}", bufs=6)
                        nc.tensor.matmul(po, vTa[:, h, m * 128:(m + 1) * 128],
                                         attWT[:, h * 96:(h + 1) * 96],
                                         start=True, stop=True)
                        dst = accR[m][:, h * 96:(h + 1) * 96]
                        if h % 2 == 0:
                            nc.vector.tensor_copy(out=dst, in_=po)
                        else:
                            nc.scalar.activation(
                                out=dst, in_=po,
                                func=mybir.ActivationFunctionType.Copy)

            # ============ Phase 4: column attention (needs exchange) ========
            q3 = q_sb.rearrange("p (h w) -> p h w", w=W)
            kf4 = kf.rearrange("p g (h w) -> p g h w", w=W)
            with (
                tc.tile_pool(name="ph4p", bufs=1) as ph4p,
                tc.tile_pool(name="cps", bufs=1, space="PSUM") as cps,
            ):
                attHT = ph4p.tile([96, NPOS], dt.bfloat16, tag="aht",
                                  name="attHT")
                mask_sb = ph4p.tile([96, NPOS], dt.bfloat16, tag="msk",
                                    name="mask_sb")
                nc.scalar.dma_start(out=mask_sb, in_=mask_d[:])
                WB = [10] * 9 + [6]     # 96 w in blocks of 10 (tail 6)
                w = 0
                for nb in WB:
                    peh = cps.tile([96, nb * HC], dt.float32, tag="peh",
                                   name=f"peh{w}", bufs=2)
                    for i in range(nb):
                        nc.tensor.matmul(peh[:, i * HC:(i + 1) * HC],
                                         kf4[:, :, :, w + i], q3[:, :, w + i],
                                         start=True, stop=True)
                    nc.scalar.activation(
                        out=attHT[:, w * HC:(w + nb) * HC], in_=peh,
                        func=mybir.ActivationFunctionType.Exp)
                    # zero the masked diagonal for this block right away
                    nc.vector.tensor_mul(
                        out=attHT[:, w * HC:(w + nb) * HC],
                        in0=attHT[:, w * HC:(w + nb) * HC],
                        in1=mask_sb[:, w * HC:(w + nb) * HC])
                    w += nb
                for w in range(96):
                    for m in range(2):
                        po2 = cps.tile([128, HC], dt.float32, tag="po2",
                                       name=f"po2{w}{m}", bufs=4)
                        nc.tensor.matmul(po2, vTb[:, w, m * 128:(m + 1) * 128],
                                         attHT[:, w * HC:(w + 1) * HC],
                                         start=True, stop=True)
                        dst = accC[m][:, w * HC:(w + 1) * HC]
                        if w % 2 == 0:
                            nc.vector.tensor_copy(out=dst, in_=po2)
                        else:
                            nc.scalar.activation(
                                out=dst, in_=po2,
                                func=mybir.ActivationFunctionType.Copy)
                # joint denominator, duplicated across all 128 partitions:
                # D[(h,w)] = (SV/gamma) * (sum_W' attWT + sum_H' attHT)
                attHT_hw = attHT.rearrange("p (w h) -> p h w", h=HC)
                h = 0
                for n, nh in enumerate([5] * 9 + [3]):  # chunks of h-rows
                    pD = cps.tile([128, nh * 96], dt.float32, tag="pD",
                                  name=f"pD{n}", bufs=2)
                    nc.tensor.matmul(pD, onesD,
                                     attWT[:, h * 96:(h + nh) * 96],
                                     start=True, stop=False)
                    nc.tensor.matmul(pD, onesD,
                                     attHT_hw[:, h:h + nh, :],
                                     start=False, stop=True)
                    with nc.allow_low_precision(reason="bf16 softmax recip"):
                        nc.vector.reciprocal(
                            out=recipDd[:, h * 96:(h + nh) * 96], in_=pD)
                    h += nh

            # ============ Phase 5: normalize, residual, out =================
            accC_hw = [accC[m].rearrange("p (w h) -> p h w", h=HC)
                       for m in range(2)]
            with (
                tc.tile_pool(name="fin", bufs=1) as finp,
            ):
                h = 0
                for cidx, nh in enumerate([5] * 9 + [3]):
                    sl = slice(h * 96, (h + nh) * 96)
                    for m in range(2):
                        xr = finp.tile([128, 480], dt.float32, tag=f"xr{m}",
                                       name=f"xr{cidx}{m}", bufs=2)
                        nc.sync.dma_start(out=xr[:, 0:nh * 96],
                                          in_=xres_d[m][:, sl])
                        nf = finp.tile([128, 480], dt.bfloat16, tag=f"nf{m}",
                                       name=f"nf{cidx}{m}", bufs=2)
                        eng = nc.vector if m == 0 else nc.gpsimd
                        eng.tensor_add(
                            out=nf[:, 0:nh * 96], in0=accR[m][:, sl],
                            in1=accC_hw[m][:, h:h + nh, :])
                        eng.tensor_mul(out=nf[:, 0:nh * 96],
                                       in0=nf[:, 0:nh * 96],
                                       in1=recipDd[:, sl])
                        fo = finp.tile([128, 480], dt.float32, tag=f"fo{m}",
                                       name=f"fo{cidx}{m}", bufs=2)
                        eng.tensor_add(out=fo[:, 0:nh * 96],
                                       in0=nf[:, 0:nh * 96], in1=xr[:, 0:nh * 96])
                        nc.sync.dma_start(out=out_d[m][:, sl],
                                          in_=fo[:, 0:nh * 96])
                    h += nh

    nc.compile()
    return nc


def _prepare_inputs(x, w_ms, b_ms, wq, bq, wk, bk, wv, bv, gamma):
    offs, taps = _fold_taps(np.asarray(w_ms, np.float32))
    x = np.asarray(x, np.float32)
    bsum = np.asarray(b_ms, np.float32).sum(0)
    gamma_f = float(np.asarray(gamma))
    bv = np.asarray(bv, np.float32)

    w25 = np.empty((128, 25, 2, 2, 128), np.float32)
    for t, off in enumerate(offs):
        # taps[off] is [co, ci]; lhsT wants [ci_lo, t, kt, m, co_lo]
        wt = taps[off].T.reshape(2, 128, 2, 128)   # [kt, ci_lo, m, co_lo]
        w25[:, t] = wt.transpose(1, 0, 2, 3)       # [ci_lo, kt, m, co_lo]
    w25 = (w25 * SW).astype(F8)
    wq8 = (np.asarray(wq, np.float32).T.reshape(2, 128, CQ) * SW) \
        .astype(F8).transpose(1, 0, 2).copy()      # [ci_lo, kt, CQ]
    wk8 = (np.asarray(wk, np.float32).T.reshape(2, 128, CQ) * SW) \
        .astype(F8).transpose(1, 0, 2).copy()
    wv8 = (np.asarray(wv, np.float32).T.reshape(2, 128, 256) * SW) \
        .astype(F8).transpose(1, 0, 2).copy()
    bq_a = np.ascontiguousarray(np.asarray(bq, np.float32).reshape(CQ, 1))
    bk_a = np.ascontiguousarray(np.asarray(bk, np.float32).reshape(CQ, 1))
    bsum_a = np.ascontiguousarray(bsum.reshape(2, 128, 1))

    in_maps = []
    for core in range(NCORES):
        b, g = core // 2, core % 2
        h0 = g * HC
        xp = np.zeros((2, 128, HP, WP), np.float32)
        xs = x[b, :, max(0, h0 - 3):h0 + HC + 3, :]     # rows with halo
        r0 = 3 if h0 == 0 else 0
        xp[:, :, r0:r0 + xs.shape[1], 3:3 + W] = \
            xs.reshape(2, 128, xs.shape[1], W)
        mask01 = np.ones((96, NPOS), np.float32)
        for h in range(HC):
            mask01[h0 + h, np.arange(96) * HC + h] = 0.0
        xres = (x[b, :, h0:h0 + HC, :].reshape(C, NPOS)
                + gamma_f * bv[:, None]).reshape(2, 128, NPOS)
        in_maps.append({
            "xpad": xp.transpose(1, 0, 2, 3).astype(F8).copy(),
            "w25": w25, "wq8": wq8, "wk8": wk8, "wv8": wv8,
            "bq": bq_a, "bk": bk_a, "bsum": bsum_a,
            "mask01": mask01.astype(BF16),
            "xres": np.ascontiguousarray(xres.astype(np.float32)),
        })
    return in_maps, gamma_f, offs


def run(inputs, trace=False):
    from concourse.bass_utils import run_bass_kernel_spmd
    in_maps, gamma_f, offs = _prepare_inputs(**inputs)
    nc = _build_program(gamma_f, offs)
    res = run_bass_kernel_spmd(nc, in_maps, list(range(NCORES)), trace=trace)
    out = np.empty((B, C, H, W), np.float32)
    for core in range(NCORES):
        b, g = core // 2, core % 2
        r = np.asarray(res.results[core]["out"]).reshape(C, HC, W)
        out[b, :, g * HC:(g + 1) * HC, :] = r
    return out, res


def kernel(**inputs) -> np.ndarray:
    out, _ = run(inputs, trace=False)
    return out
